# revision 10
# baseline (speedup 1.0000x reference)
"""DeepGraphSAGE on 8 Trainium2 NeuronCores via Bass/Tile.

Strategy (graph/data parallel, per sharding hint):
- Nodes sharded across 8 cores (12500 rows each); edges partitioned by
  destination core.
- Per layer: every core gathers source features h[src] for its edges from a
  replicated bf16 node-major table via dma_gather, segment-sums them on the
  TensorEngine (one-hot "S" matrices built on DVE, PSUM accumulation over
  128-destination windows, 1/deg folded into S), applies the two 128x128
  linears feature-major on the PE, computes BatchNorm batch stats locally,
  AllReduces them, applies the fused scale/bias + ReLU on the scalar engine,
  and AllGathers the new bf16 table for the next layer.
- b_l is dropped: a per-feature constant shift is exactly cancelled by
  training-mode BatchNorm.
"""

import numpy as np

import concourse.bass as bass
import concourse.bacc as bacc
import concourse.mybir as mybir
from concourse import tile

F32 = mybir.dt.float32
BF16 = mybir.dt.float16  # message-path dtype (fp16: full PE rate, 10-bit mantissa)
I16 = mybir.dt.int16

NCORES = 8
D = 128
NCHUNK = 4  # gather-table chunks (int16 index limit: chunk rows <= 32767)
MAXB = 8  # blocks per gather call (SWDGE ring holds 1024 descriptors)
BN_EPS = 1e-5


def _wrap_idxs(arr):
    """int16 idx array (len % 128 == 0) -> [128, len/16] wrapped layout."""
    n = arr.shape[0]
    w = arr.reshape(n // 16, 16).T.astype(np.int16)
    return np.tile(w, (8, 1))


def _preprocess(x, edge_index, n_nodes, n_cores):
    """Partition edges by destination core; build the static block layout
    (identical across cores via cross-core max padding) and per-core arrays."""
    N = n_nodes
    SH = N // n_cores
    NW = (SH + 127) // 128
    CH = N // NCHUNK
    assert CH <= 32767 and N % NCHUNK == 0 and N % n_cores == 0

    src = np.asarray(edge_index[0], dtype=np.int64)
    dst = np.asarray(edge_index[1], dtype=np.int64)
    deg = np.bincount(dst, minlength=N)

    c_of = dst // SH
    ld = dst - c_of * SH
    w_of = ld >> 7
    d_local = ld & 127
    k_of = src // CH
    i16 = (src - k_of * CH).astype(np.int16)

    cnt = np.bincount(
        (c_of * NW + w_of) * NCHUNK + k_of, minlength=n_cores * NW * NCHUNK
    ).reshape(n_cores, NW, NCHUNK)
    B = -(-cnt.max(axis=0) // 128)  # [NW, NCHUNK] blocks, cross-core max

    # block layout order: for k -> for w; gather calls are <=MAXB-block
    # slices of each k-run (SWDGE ring limit: 1024 descriptors per call)
    layout = []  # ((w, k), nblocks) in layout order
    calls = []  # (k, block_start, nblocks) per gather call
    for k in range(NCHUNK):
        bs = sum(b for _, b in layout)
        for w in range(NW):
            layout.append(((w, k), int(B[w, k])))
        nbk = int(B[:, k].sum())
        off = bs
        while off < bs + nbk:
            nb = min(MAXB, bs + nbk - off)
            calls.append((k, off, nb))
            off += nb
    TB = sum(b for _, b in layout)
    col_off = {}
    off = 0
    for (w, k), b in layout:
        col_off[(w, k)] = off
        off += b

    # per-edge position within its per-core padded layout
    order = np.lexsort((k_of, w_of, c_of))
    so_c, so_w, so_k = c_of[order], w_of[order], k_of[order]
    key = (so_c * NW + so_w) * NCHUNK + so_k
    first = np.ones(len(key), dtype=bool)
    first[1:] = key[1:] != key[:-1]
    grp_start = np.maximum.accumulate(np.where(first, np.arange(len(key)), 0))
    rank = np.arange(len(key)) - grp_start
    co_arr = np.zeros((NW, NCHUNK), dtype=np.int64)
    for (w, k), _ in layout:
        co_arr[w, k] = col_off[(w, k)]
    pos = co_arr[so_w, so_k] * 128 + rank

    idx_flat = np.zeros((n_cores, TB * 128), dtype=np.int16)
    dl_flat = np.full((n_cores, TB * 128), -1.0, dtype=np.float32)
    dg_flat = np.ones((n_cores, TB * 128), dtype=np.float32)
    idx_flat[so_c, pos] = i16[order]
    dl_flat[so_c, pos] = d_local[order]
    dg_flat[so_c, pos] = deg[dst[order]]

    # idx DRAM layout: per-call wrapped, concatenated along free dim
    idx_per_core = [
        np.concatenate(
            [
                _wrap_idxs(idx_flat[c, bs * 128 : (bs + nb) * 128])
                for (_, bs, nb) in calls
            ],
            axis=1,
        )
        for c in range(n_cores)
    ]
    dl_per_core = [
        np.ascontiguousarray(dl_flat[c].reshape(TB, 128).T) for c in range(n_cores)
    ]
    dg_per_core = [
        np.ascontiguousarray(dg_flat[c].reshape(TB, 128).T) for c in range(n_cores)
    ]

    meta = dict(
        N=N, SH=SH, NW=NW, CH=CH, TB=TB, calls=calls, B=B,
        col_off=col_off, layout=layout,
    )
    return meta, idx_per_core, dl_per_core, dg_per_core


def _build(meta, n_layers, d_out):
    """Build the SPMD Bass program (identical for all cores)."""
    N, SH, NW, CH, TB = meta["N"], meta["SH"], meta["NW"], meta["CH"], meta["TB"]
    calls, B = meta["calls"], meta["B"]
    col_off = meta["col_off"]
    col_info = {}
    for (w, k), nb in meta["layout"]:
        for bi in range(nb):
            col_info[col_off[(w, k)] + bi] = (w, bi == 0, bi == nb - 1)
    SHP = NW * 128  # padded shard columns
    L = n_layers
    n_ag = L - 1  # AllGathers needed (layers 1..L-1 gather new tables)
    NT = (SHP + 511) // 512  # linear node tiles

    nc = bacc.Bacc(
        "TRN2",
        target_bir_lowering=False,
        debug=False,
        enable_asserts=True,
        num_devices=NCORES,
    )

    # ---- I/O ----
    xtab = nc.dram_tensor("xtab", [N, D], BF16, kind="ExternalInput")
    xT = nc.dram_tensor("xT", [D, SHP], F32, kind="ExternalInput")
    idxd = nc.dram_tensor("idxd", [128, TB * 8], I16, kind="ExternalInput")
    dld = nc.dram_tensor("dld", [128, TB], F32, kind="ExternalInput")
    dgd = nc.dram_tensor("dgd", [128, TB], F32, kind="ExternalInput")
    wld = nc.dram_tensor("wld", [L, D, D], BF16, kind="ExternalInput")
    wrd = nc.dram_tensor("wrd", [L, D, D], F32, kind="ExternalInput")
    gmd = nc.dram_tensor("gmd", [D, L], F32, kind="ExternalInput")
    btd = nc.dram_tensor("btd", [D, L], F32, kind="ExternalInput")
    wod = nc.dram_tensor("wod", [D, d_out], F32, kind="ExternalInput")
    bod = nc.dram_tensor("bod", [1, d_out], F32, kind="ExternalInput")
    onesd = nc.dram_tensor("onesd", [1, 128], F32, kind="ExternalInput")
    idend = nc.dram_tensor("idend", [D, D], F32, kind="ExternalInput")
    iotad = nc.dram_tensor("iotad", [D, D], BF16, kind="ExternalInput")
    outd = nc.dram_tensor("outd", [SH, d_out], F32, kind="ExternalOutput")

    # collective buffers
    agin = [
        nc.dram_tensor(f"agin{l}", [SH, D], BF16, kind="Internal")
        for l in range(n_ag)
    ]
    tables = [
        nc.dram_tensor(
            f"table{l}", [N, D], BF16, kind="Internal", addr_space="Shared"
        )
        for l in range(n_ag)
    ]
    bnin = [
        nc.dram_tensor(f"bnin{l}", [D, 2], F32, kind="Internal")
        for l in range(L)
    ]
    bnout = [
        nc.dram_tensor(f"bnout{l}", [D, 2], F32, kind="Internal", addr_space="Shared")
        for l in range(L)
    ]

    rg = [list(range(NCORES))]

    with tile.TileContext(nc) as tc:
        with (
            tc.tile_pool(name="big", bufs=1) as big,
            tc.tile_pool(name="wts", bufs=1) as wts,
            tc.tile_pool(name="msgp", bufs=3) as msgp,
            tc.tile_pool(name="sp", bufs=4) as sp,
            tc.tile_pool(name="castp", bufs=4) as castp,
            tc.tile_pool(name="outp", bufs=4) as outp,
            tc.tile_pool(name="statp", bufs=1) as statp,
            tc.tile_pool(name="pagg", bufs=3, space="PSUM") as pagg,
            tc.tile_pool(name="plin", bufs=2, space="PSUM") as plin,
        ):
            # ---- persistent SBUF state ----
            h0 = big.tile([D, SHP], F32, tag="h0")
            h1 = big.tile([D, SHP], F32, tag="h1")
            aggT = big.tile([D, SHP], BF16, tag="aggT")
            dl_t = big.tile([128, TB], F32, tag="dl")
            inv_t = big.tile([128, TB], F32, tag="inv")
            idx_t = big.tile([128, TB * 8], I16, tag="idxt")
            iota_t = wts.tile([D, D], BF16, tag="iota")
            iden_t = wts.tile([D, D], F32, tag="iden")
            wl_t = [wts.tile([D, D], BF16, tag=f"wl{l}", name=f"wl_t{l}") for l in range(L)]
            wr_t = [wts.tile([D, D], F32, tag=f"wr{l}", name=f"wr_t{l}") for l in range(L)]
            gm_t = wts.tile([D, L], F32, tag="gm")
            bt_t = wts.tile([D, L], F32, tag="bt")
            wo_t = wts.tile([D, d_out], F32, tag="wo")
            bo_t = wts.tile([1, d_out], F32, tag="bo")
            ones_t = wts.tile([1, 128], F32, tag="ones")

            # ---- load constants ----
            nc.sync.dma_start(h0[:], xT[:])
            nc.sync.dma_start(dl_t[:], dld[:])
            nc.sync.dma_start(idx_t[:], idxd[:])
            nc.sync.dma_start(iota_t[:], iotad[:])
            nc.sync.dma_start(iden_t[:], idend[:])
            for l in range(L):
                nc.sync.dma_start(wl_t[l][:], wld[l, :, :])
                nc.sync.dma_start(wr_t[l][:], wrd[l, :, :])
            nc.sync.dma_start(gm_t[:], gmd[:])
            nc.sync.dma_start(bt_t[:], btd[:])
            nc.sync.dma_start(wo_t[:], wod[:])
            nc.sync.dma_start(bo_t[:], bod[:])
            nc.sync.dma_start(ones_t[:], onesd[:])

            # inv_deg = 1 / deg (deg >= 1 everywhere by construction)
            nc.sync.dma_start(inv_t[:], dgd[:])
            nc.vector.reciprocal(inv_t[:], inv_t[:])

            # BN stat scratch
            ssum = statp.tile([D, 1], F32, tag="ssum")
            ssq = statp.tile([D, 1], F32, tag="ssq")
            bns = statp.tile([D, 2], F32, tag="bns")
            bns2 = statp.tile([D, 2], F32, tag="bns2")
            mean_t = statp.tile([D, 1], F32, tag="mean")
            ex2_t = statp.tile([D, 1], F32, tag="ex2")
            var_t = statp.tile([D, 1], F32, tag="var")
            rstd_t = statp.tile([D, 1], F32, tag="rstd")
            a_t = statp.tile([D, 1], F32, tag="a")
            b_t = statp.tile([D, 1], F32, tag="b")
            nb_t = statp.tile([D, 1], F32, tag="nb")

            h_bufs = [h0, h1]
            for l in range(L):
                hT = h_bufs[l % 2]
                hN = h_bufs[(l + 1) % 2]
                tbl = xtab if l == 0 else tables[l - 1]

                # ---- edge aggregation into aggT ----
                # gather calls are <=MAXB blocks (SWDGE ring limit); each
                # (window,chunk) bucket accumulates on the PE into PSUM
                # (possibly across call boundaries), then is copied/added
                # into the SBUF accumulator aggT.
                first_bucket = [True] * NW
                open_pt = {}
                for k, cs, nb in calls:
                    mg = msgp.tile([128, nb, D], BF16, tag="msg", name="mg")
                    nc.gpsimd.dma_gather(
                        mg[:],
                        tbl[k * CH : (k + 1) * CH, :],
                        idx_t[:, cs * 8 : (cs + nb) * 8],
                        nb * 128,
                        nb * 128,
                        D,
                    )
                    for j in range(nb):
                        col = cs + j
                        w, isf, isl = col_info[col]
                        if isf:
                            open_pt[w] = pagg.tile(
                                [D, 128], F32, tag="pagg", name="pt"
                            )
                        pt = open_pt[w]
                        st = sp.tile([128, 128], BF16, tag="s", name="st")
                        nc.vector.tensor_scalar(
                            st[:],
                            iota_t[:],
                            dl_t[:, col : col + 1],
                            inv_t[:, col : col + 1],
                            op0=mybir.AluOpType.is_equal,
                            op1=mybir.AluOpType.mult,
                        )
                        nc.tensor.matmul(
                            pt[:], mg[:, j, :], st[:], start=isf, stop=isl
                        )
                        if isl:
                            del open_pt[w]
                            dstap = aggT[:, w * 128 : (w + 1) * 128]
                            if first_bucket[w]:
                                nc.vector.tensor_copy(dstap, pt[:])
                                first_bucket[w] = False
                            else:
                                nc.vector.tensor_tensor(
                                    dstap, dstap, pt[:], op=mybir.AluOpType.add
                                )

                # ---- linear: hN = W_l.T @ aggT + W_r.T @ hT ----
                for t in range(NT):
                    t0 = t * 512
                    tw = min(512, SHP - t0)
                    pl = plin.tile([D, 512], F32, tag="plin")
                    nc.tensor.matmul(
                        pl[:, :tw],
                        wl_t[l][:],
                        aggT[:, t0 : t0 + tw],
                        start=True,
                        stop=False,
                    )
                    nc.tensor.matmul(
                        pl[:, :tw],
                        wr_t[l][:],
                        hT[:, t0 : t0 + tw],
                        start=False,
                        stop=True,
                    )
                    nc.vector.tensor_copy(hN[:, t0 : t0 + tw], pl[:, :tw])

                # ---- BN stats (over real nodes only) + AllReduce ----
                nc.vector.reduce_sum(
                    ssum[:], hN[:, :SH], axis=mybir.AxisListType.X
                )
                nc.scalar.activation(
                    aggT[:, :SH],
                    hN[:, :SH],
                    mybir.ActivationFunctionType.Square,
                    accum_out=ssq[:],
                )
                nc.vector.tensor_copy(bns[:, 0:1], ssum[:])
                nc.vector.tensor_copy(bns[:, 1:2], ssq[:])
                nc.sync.dma_start(bnin[l][:], bns[:])
                nc.gpsimd.collective_compute(
                    "AllReduce",
                    mybir.AluOpType.add,
                    replica_groups=rg,
                    ins=[bnin[l][:]],
                    outs=[bnout[l][:]],
                )
                nc.sync.dma_start(bns2[:], bnout[l][:])

                # a = gamma * rsqrt(var + eps); b = beta - mean * a
                inv_n = 1.0 / float(N)
                nc.vector.tensor_scalar(
                    bns2[:], bns2[:], inv_n, None, op0=mybir.AluOpType.mult
                )
                nc.vector.tensor_copy(mean_t[:], bns2[:, 0:1])
                nc.vector.tensor_copy(ex2_t[:], bns2[:, 1:2])
                nc.vector.tensor_tensor(
                    var_t[:], mean_t[:], mean_t[:], op=mybir.AluOpType.mult
                )
                nc.vector.tensor_tensor(
                    var_t[:], ex2_t[:], var_t[:], op=mybir.AluOpType.subtract
                )
                nc.vector.tensor_scalar(
                    var_t[:], var_t[:], BN_EPS, None, op0=mybir.AluOpType.add
                )
                nc.scalar.activation(
                    rstd_t[:], var_t[:], mybir.ActivationFunctionType.Sqrt
                )
                nc.vector.reciprocal(rstd_t[:], rstd_t[:])
                nc.vector.tensor_tensor(
                    a_t[:], rstd_t[:], gm_t[:, l : l + 1], op=mybir.AluOpType.mult
                )
                nc.vector.tensor_tensor(
                    nb_t[:], mean_t[:], a_t[:], op=mybir.AluOpType.mult
                )
                nc.vector.tensor_tensor(
                    b_t[:], bt_t[:, l : l + 1], nb_t[:], op=mybir.AluOpType.subtract
                )
                # hN = relu(a * hN + b)
                nc.scalar.activation(
                    hN[:],
                    hN[:],
                    mybir.ActivationFunctionType.Relu,
                    bias=b_t[:],
                    scale=a_t[:],
                )

                # ---- publish bf16 node-major table for next layer ----
                if l < L - 1:
                    for t in range(NW):
                        t0 = t * 128
                        rows = min(128, SH - t0)
                        ptr = plin.tile([128, 128], F32, tag="plin")
                        nc.tensor.transpose(
                            ptr[:], hN[:, t0 : t0 + 128], iden_t[:]
                        )
                        hb = castp.tile([128, 128], BF16, tag="cast")
                        nc.vector.tensor_copy(hb[:], ptr[:])
                        nc.sync.dma_start(
                            agin[l][t0 : t0 + rows, :], hb[:rows, :]
                        )
                    nc.gpsimd.collective_compute(
                        "AllGather",
                        mybir.AluOpType.bypass,
                        replica_groups=rg,
                        ins=[agin[l][:]],
                        outs=[tables[l][:]],
                    )

            # ---- final projection: out = h.T @ W_o + b_o (node-major) ----
            hF = h_bufs[L % 2]
            for t in range(NW):
                t0 = t * 128
                rows = min(128, SH - t0)
                pf = plin.tile([128, 512], F32, tag="plin")
                nc.tensor.matmul(
                    pf[:, :d_out], ones_t[:], bo_t[:], start=True, stop=False
                )
                nc.tensor.matmul(
                    pf[:, :d_out],
                    hF[:, t0 : t0 + 128],
                    wo_t[:],
                    start=False,
                    stop=True,
                )
                ot = outp.tile([128, d_out], F32, tag="ot")
                nc.vector.tensor_copy(ot[:], pf[:, :d_out])
                nc.sync.dma_start(outd[t0 : t0 + rows, :], ot[:rows, :])

    nc.compile()
    return nc


def _make_runner(nc, n_cores=NCORES):
    """Build a reusable jitted SPMD callable (mirrors bass2jax.run_bass_via_pjrt
    multi-core path, without output donation so repeat calls work)."""
    import jax
    from concourse import bass2jax as b2j

    b2j.install_neuronx_cc_hook()
    partition_name = (
        nc.partition_id_tensor.name if nc.partition_id_tensor is not None else None
    )
    in_names, out_names, out_avals, zero_outs = [], [], [], []
    for alloc in nc.m.functions[0].allocations:
        if not isinstance(alloc, mybir.MemoryLocationSet):
            continue
        name = alloc.memorylocations[0].name
        if alloc.kind == "ExternalInput":
            if name != partition_name:
                in_names.append(name)
        elif alloc.kind == "ExternalOutput":
            shape = tuple(alloc.tensor_shape)
            dtype = mybir.dt.np(alloc.dtype)
            out_names.append(name)
            out_avals.append(jax.core.ShapedArray(shape, dtype))
            zero_outs.append(np.zeros(shape, dtype))
    n_params = len(in_names)
    all_in = list(in_names) + list(out_names)
    if partition_name is not None:
        all_in.append(partition_name)

    def _body(*args):
        operands = list(args)
        if partition_name is not None:
            operands.append(b2j.partition_id_tensor())
        outs = b2j._bass_exec_p.bind(
            *operands,
            out_avals=tuple(out_avals),
            in_names=tuple(all_in),
            out_names=tuple(out_names),
            lowering_input_output_aliases=(),
            sim_require_finite=True,
            sim_require_nnan=True,
            nc=nc,
        )
        return tuple(outs)

    devices = jax.devices()[:n_cores]
    mesh = b2j.Mesh(np.asarray(devices), ("core",))
    pspec = b2j.PartitionSpec("core")
    fn = jax.jit(
        b2j.shard_map(
            _body,
            mesh=mesh,
            in_specs=(pspec,) * (n_params + len(out_names)),
            out_specs=(pspec,) * len(out_names),
            check_rep=False,
        ),
        keep_unused=True,
    )

    class Runner:
        pass

    r = Runner()
    r.fn = fn
    r.mesh = mesh
    r.in_names = in_names
    r.out_names = out_names
    r.out_avals = out_avals
    r.zero_outs = zero_outs
    r.n_cores = n_cores

    def prepare_args(in_maps):
        concat_in = [
            np.concatenate(
                [np.asarray(in_maps[c][name]) for c in range(n_cores)], axis=0
            )
            for name in in_names
        ]
        concat_zero = [
            np.zeros((n_cores * z.shape[0], *z.shape[1:]), z.dtype)
            for z in zero_outs
        ]
        return [*concat_in, *concat_zero]

    def run(args):
        import jax

        out_arrs = fn(*args)
        jax.block_until_ready(out_arrs)
        return [
            {
                name: np.asarray(out_arrs[i]).reshape(
                    n_cores, *out_avals[i].shape
                )[c]
                for i, name in enumerate(out_names)
            }
            for c in range(n_cores)
        ]

    r.prepare_args = prepare_args
    r.run = run
    return r


def _prepare(x, edge_index, W_l, b_l, W_r, gamma, beta, W_o, b_o, n_cores=NCORES):
    N, d_in = x.shape
    L = W_l.shape[0]
    d_out = W_o.shape[1]
    SH = N // n_cores

    meta, idx_pc, dl_pc, dg_pc = _preprocess(x, edge_index, N, n_cores)
    SHP = meta["NW"] * 128

    nc = _build(meta, L, d_out)

    xf = np.asarray(x, dtype=np.float32)
    xtab = xf.astype(np.float16)
    wl = np.asarray(W_l, np.float32).astype(np.float16)
    wr = np.asarray(W_r, np.float32)
    gm = np.asarray(gamma, np.float32).T.copy()  # [D, L]
    bt = np.asarray(beta, np.float32).T.copy()
    wo = np.asarray(W_o, np.float32)
    bo = np.asarray(b_o, np.float32).reshape(1, -1)
    ones = np.ones((1, 128), np.float32)
    iden = np.eye(128, dtype=np.float32)
    iota = np.tile(np.arange(128, dtype=np.float32), (128, 1)).astype(np.float16)

    in_maps = []
    for c in range(n_cores):
        xs = xf[c * SH : (c + 1) * SH].T  # [D, SH]
        xsp = np.zeros((D, SHP), np.float32)
        xsp[:, :SH] = xs
        in_maps.append(
            {
                "xtab": xtab,
                "xT": xsp,
                "idxd": idx_pc[c],
                "dld": dl_pc[c],
                "dgd": dg_pc[c],
                "wld": wl,
                "wrd": wr,
                "gmd": gm,
                "btd": bt,
                "wod": wo,
                "bod": bo,
                "onesd": ones,
                "idend": iden,
                "iotad": iota,
            }
        )

    runner = _make_runner(nc, n_cores)
    return runner, in_maps


def _assemble(results, n_cores=NCORES):
    return np.concatenate(
        [results[c]["outd"] for c in range(n_cores)], axis=0
    )


def kernel(**inputs):
    runner, in_maps = _prepare(
        inputs["x"],
        inputs["edge_index"],
        inputs["W_l"],
        inputs["b_l"],
        inputs["W_r"],
        inputs["gamma"],
        inputs["beta"],
        inputs["W_o"],
        inputs["b_o"],
    )
    results = runner.run(runner.prepare_args(in_maps))
    return _assemble(results)
